# revision 18
# baseline (speedup 1.0000x reference)
"""Multi-head attention block (B=16, N=1024, D=768, H=12) on 8 TRN2 NeuronCores.

Strategy: pure data parallelism - 2 batch items per core, no collectives.
Host pre-transposes x to x^T, pre-arranges W_qkv's q|k columns into the
SBUF-resident [partition, nt, kt, col] layout (so every weight DMA moves
contiguous 1.5KB bursts), and casts operands to bf16.

The ScalarE exp stream (25.2M elems/core, ~214us) and the TensorE matmul
stream (~246us of moving cycles) are near-equal floors. The engine queues
are strict FIFO, so the kernel is built to transmit the ACT pace to the PE
at exactly one point per iteration and never expose a wait anywhere else:
  - scores [keys, queries] per (head-pair, query-half hf, key-tile kt):
    two 64-row-group matmuls run concurrently; exp on ScalarE with fused
    1/sqrt(hd) scale; a ones column per head makes the PV matmul also
    produce the softmax denominators.
  - PV matmuls are software-pipelined TWO iterations behind their exp, so
    they never wait on the ACT engine; only the scores' PSUM-pool rotation
    (2-deep) throttles the PE to the ACT cadence.
  - all non-attention matmuls (QKV columns, V halves, output projection)
    are chopped into single-matmul steps and dripped into each iteration
    by a per-iteration PE-slack budget; require() force-drains any steps
    a pair's inputs depend on before the pair is emitted (the Tile
    framework has program-order semantics).
  - V production is split into j-halves: head-pairs 0-2 need only the
    j=0 half, so pair 0 can start after 4 qk units + a few v half-units.
  - b1's projection is split: 4 row-blocks run inside the last attention
    window, 4 in the tail.
"""

import sys
import types
import numpy as np
import ml_dtypes
from collections import deque
from contextlib import ExitStack

# --- shim: provide antenv.axon_hooks so trace=True works under axon ---
if "antenv.axon_hooks" not in sys.modules:
    try:
        from trn_agent_boot.trn_boot import _ntff_profile_via_ctypes

        _hooks_mod = types.ModuleType("antenv.axon_hooks")
        _ntff_hook = _ntff_profile_via_ctypes("/opt/axon/libaxon_pjrt.so")
        _hooks_mod.get_axon_ntff_profile_hook = lambda: _ntff_hook
        _hooks_mod.set_axon_ntff_profile_hook = lambda h: None
        sys.modules["antenv.axon_hooks"] = _hooks_mod
    except Exception:
        pass

import concourse.bass as bass
import concourse.tile as tile
from concourse import bacc, mybir
import concourse.bass_utils as bass_utils
from concourse.bass_utils import run_bass_kernel_spmd

bass_utils.upload_artifacts = lambda tmpdir: tmpdir  # no S3 in sandbox

F32 = mybir.dt.float32
BF16 = mybir.dt.bfloat16
EXP = mybir.ActivationFunctionType.Exp
MULT = mybir.AluOpType.mult

NCORES = 8
B, N, D = 16, 1024, 768
H, HD = 12, 64
BPC = B // NCORES        # batch items per core
ROWS = BPC * N           # 2048
P = 128
KT = D // P              # 6 contraction tiles
NKT = N // P             # 8 attention key tiles
NP = H // 2              # 6 head pairs
SCALE = HD ** -0.5

SLACK_PER_ITER = 560.0   # PE slack per ACT-paced iteration (ns)
BUDGET_CAP = 2000.0
MM_QK = 215.0            # one 512-moving matmul
MM_VP = 165.0            # one 384-moving matmul


def build_kernel():
    nc = bacc.Bacc("TRN2", target_bir_lowering=False, debug=False, num_devices=NCORES)
    xT = nc.dram_tensor("xT", [D, ROWS], BF16, kind="ExternalInput").ap()
    # host-prearranged: wqk[p, nt*KT*P + kt*P + c] = W_qkv[kt*P + p, nt*P + c]
    wqk = nc.dram_tensor("wqk", [P, 2 * KT * KT * P], BF16, kind="ExternalInput").ap()
    wv = nc.dram_tensor("wv", [D, D], BF16, kind="ExternalInput").ap()
    wproj = nc.dram_tensor("wproj", [D, D], BF16, kind="ExternalInput").ap()
    bias = nc.dram_tensor("bias", [P, D], F32, kind="ExternalInput").ap()
    out = nc.dram_tensor("out", [ROWS, D], F32, kind="ExternalOutput").ap()

    with tile.TileContext(nc) as tc, ExitStack() as ctx:
        const = ctx.enter_context(tc.tile_pool(name="const", bufs=1))
        xp = ctx.enter_context(tc.tile_pool(name="xT", bufs=2))
        qkp = ctx.enter_context(tc.tile_pool(name="qkT", bufs=2))
        vp = ctx.enter_context(tc.tile_pool(name="v", bufs=2))
        aop = ctx.enter_context(tc.tile_pool(name="ao", bufs=2))
        exp_p = ctx.enter_context(tc.tile_pool(name="exp", bufs=4))
        smallp = ctx.enter_context(tc.tile_pool(name="small", bufs=2))
        yp = ctx.enter_context(tc.tile_pool(name="y", bufs=3))
        ps_sc = ctx.enter_context(tc.tile_pool(name="ps_sc", bufs=2, space="PSUM"))
        ps_out = ctx.enter_context(tc.tile_pool(name="ps_out", bufs=2, space="PSUM"))
        ps_mm = ctx.enter_context(tc.tile_pool(name="ps_mm", bufs=2, space="PSUM"))

        # warm the ACT exp table set during the DMA lead-in
        warm = smallp.tile([1, 16], F32, tag="warm")
        nc.vector.memset(warm[:], 0.0)
        warm2 = smallp.tile([1, 16], BF16, tag="warm2")
        nc.scalar.activation(warm2[:], warm[:], EXP, scale=1.0)

        # --- resident weights / activations, DMA'd in priority order ---
        wqk_sb = const.tile([P, 2 * KT, KT, P], BF16)   # [p, nt, kt, c]
        wv_sb = const.tile([P, KT, D], BF16)
        wproj_sb = const.tile([P, KT, D], BF16)
        bias_sb = const.tile([P, D], F32)
        xT_ts = [xp.tile([P, KT, N], BF16, tag="xT", name=f"xT_{b}") for b in range(BPC)]
        qkT_ts = [qkp.tile([P, 2 * KT, N], BF16, tag="qkT", name=f"qkT_{b}") for b in range(BPC)]
        ao_ts = [aop.tile([P, KT, N], BF16, tag="ao", name=f"ao_{b}") for b in range(BPC)]

        def dma_wqk(nt):
            nc.sync.dma_start(
                wqk_sb[:, nt, :, :],
                wqk[:, nt * KT * P:(nt + 1) * KT * P].rearrange(
                    "p (a n) -> p a n", a=KT),
            )

        def dma_xT(b, kt, h):
            nc.sync.dma_start(
                xT_ts[b][:, kt, h * 512:(h + 1) * 512],
                xT[kt * P:(kt + 1) * P, b * N + h * 512:b * N + (h + 1) * 512],
            )

        # first chunks: enough for qk_pair(0, p=0) and early v halves
        dma_wqk(0)
        dma_wqk(KT)
        for kt in range(KT):
            dma_xT(0, kt, 0)
        # a few warm matmuls against the first weight chunk (HAM warm-up)
        for w in range(10):
            pmw = ps_mm.tile([P, 512], F32, tag="pm", name=f"pmw_{w}")
            nc.tensor.matmul(
                pmw[:, :256], wqk_sb[:, 0, 0, :], wqk_sb[:, 0, 0:2, :],
                start=True, stop=True,
            )
        for kt in range(KT):
            nc.sync.dma_start(wv_sb[:, kt, 0:384], wv[kt * P:(kt + 1) * P, 0:384])
        for kt in range(KT):
            dma_xT(0, kt, 1)
        for kt in range(KT):
            nc.sync.dma_start(wv_sb[:, kt, 384:768], wv[kt * P:(kt + 1) * P, 384:768])
        for p_ in range(1, KT):
            dma_wqk(p_)
            dma_wqk(KT + p_)
        for kt in range(KT):
            dma_xT(1, kt, 0)
            dma_xT(1, kt, 1)
        nc.sync.dma_start(wproj_sb[:], wproj.rearrange("(a p) n -> p a n", p=P))
        nc.sync.dma_start(bias_sb[:], bias)

        # v tiles carry a ones column per head: PV also produces denominators
        v_ts = []
        for b in range(BPC):
            v_flat = vp.tile([P, NKT, H * (HD + 1)], BF16, tag="v", name=f"v_{b}")
            v_t = v_flat[:].rearrange("q a (h c) -> q a h c", h=H)
            nc.vector.memset(v_t[:, :, :, HD:HD + 1], 1.0)
            v_ts.append(v_t)

        # --- work units as single-matmul steps ---
        uid = {"n": 0}

        def fresh(tag):
            uid["n"] += 1
            return f"{tag}_{uid['n']}"

        def qk_steps(b, nt, hf):
            st = {}
            def step(kt):
                def f():
                    if kt == 0:
                        st["pm"] = ps_mm.tile([P, 512], F32, tag="pm",
                                              name=fresh("pmqk"))
                    nc.tensor.matmul(
                        st["pm"][:],
                        wqk_sb[:, nt, kt, :],
                        xT_ts[b][:, kt, hf * 512:(hf + 1) * 512],
                        start=(kt == 0), stop=(kt == KT - 1),
                    )
                    if kt == KT - 1:
                        nc.vector.tensor_copy(
                            qkT_ts[b][:, nt, hf * 512:(hf + 1) * 512], st["pm"][:]
                        )
                return f
            return [step(kt) for kt in range(KT)]

        def v_steps(b, rt, j):
            st = {}
            def step(kt):
                def f():
                    if kt == 0:
                        st["pm"] = ps_mm.tile([P, 512], F32, tag="pm",
                                              name=fresh("pmv"))
                    nc.tensor.matmul(
                        st["pm"][:, :384],
                        xT_ts[b][:, kt, rt * P:(rt + 1) * P],
                        wv_sb[:, kt, j * 384:(j + 1) * 384],
                        start=(kt == 0), stop=(kt == KT - 1),
                    )
                    if kt == KT - 1:
                        nc.vector.tensor_copy(
                            v_ts[b][:, rt, j * 6:(j + 1) * 6, 0:HD],
                            st["pm"][:, :384],
                        )
                return f
            return [step(kt) for kt in range(KT)]

        def proj_steps(b, rt):
            rows0 = b * N
            st = {}
            def step(j, kt):
                def f():
                    if j == 0 and kt == 0:
                        st["y"] = yp.tile([P, D], F32, tag="y",
                                          name=fresh("ypj"))
                    if kt == 0:
                        st["pm"] = ps_mm.tile([P, 512], F32, tag="pm",
                                              name=fresh("pmpj"))
                    nc.tensor.matmul(
                        st["pm"][:, :384],
                        ao_ts[b][:, kt, rt * P:(rt + 1) * P],
                        wproj_sb[:, kt, j * 384:(j + 1) * 384],
                        start=(kt == 0), stop=(kt == KT - 1),
                    )
                    if kt == KT - 1:
                        nc.vector.tensor_add(
                            st["y"][:, j * 384:(j + 1) * 384], st["pm"][:, :384],
                            bias_sb[:, j * 384:(j + 1) * 384],
                        )
                        if j == 1:
                            nc.sync.dma_start(
                                out[rows0 + rt * P:rows0 + (rt + 1) * P, :],
                                st["y"][:],
                            )
                return f
            return [step(j, kt) for j in range(2) for kt in range(KT)]

        def emit_unit(steps):
            for s in steps:
                s()

        # --- filler queue (step granularity) + require() ordering guard ---
        filler_q = deque()  # (pe_cost, key, fn)
        state = {"budget": 0.0}

        def push_unit(key, steps, cost):
            for s in steps:
                filler_q.append((cost, key, s))

        def run_fillers():
            state["budget"] = min(state["budget"] + SLACK_PER_ITER, BUDGET_CAP)
            while filler_q and state["budget"] >= filler_q[0][0]:
                cost, _, fn = filler_q.popleft()
                state["budget"] -= cost
                fn()

        def require(keys):
            while any(it[1] in keys for it in filler_q):
                _, _, fn = filler_q.popleft()
                fn()

        def force_drain():
            while filler_q:
                _, _, fn = filler_q.popleft()
                fn()

        # --- attention: PV pipelined 2 iterations behind exp ---
        pv_q = deque()

        def pump_pv(force=False):
            while pv_q and (force or len(pv_q) > 2):
                pv_q.popleft()()

        ones_row = const.tile([1, P], BF16)
        nc.vector.memset(ones_row[:], 1.0)

        def epilogue(b, p, hf, po, fast=False):
            ao_t = ao_ts[b]
            for hs in range(2):
                u65 = smallp.tile([HD + 1, 512], F32, tag="u65")
                nc.vector.tensor_copy(u65[:], po[hs][:])
                sums_t = smallp.tile([1, 512], F32, tag="sums")
                nc.vector.tensor_copy(sums_t[:], u65[HD:HD + 1, :])
                if fast:
                    # low-latency tail variant: reciprocal on the [1,512] row,
                    # broadcast via a 1-partition PE matmul instead of GpSimd
                    rec1 = smallp.tile([1, 512], F32, tag="rec1")
                    nc.vector.reciprocal_approx_fast(rec1[:], sums_t[:])
                    recb = smallp.tile([1, 512], BF16, tag="recb")
                    nc.vector.tensor_copy(recb[:], rec1[:])
                    pmx = ps_mm.tile([P, 512], F32, tag="pm",
                                     name=fresh("pmbc"))
                    nc.tensor.matmul(pmx[:], ones_row[:], recb[:],
                                     start=True, stop=True)
                    nc.vector.tensor_tensor(
                        ao_t[hs * HD:(hs + 1) * HD, p, hf * 512:(hf + 1) * 512],
                        u65[0:HD, :], pmx[0:HD, :], MULT,
                    )
                    continue
                rbc = smallp.tile([HD, 512], F32, tag="rbc")
                nc.gpsimd.partition_broadcast(rbc[:], sums_t[:])
                rec = smallp.tile([HD, 512], F32, tag="rec")
                nc.vector.reciprocal_approx_fast(rec[:], rbc[:])
                nc.vector.tensor_tensor(
                    ao_t[hs * HD:(hs + 1) * HD, p, hf * 512:(hf + 1) * 512],
                    u65[0:HD, :], rec[:], MULT,
                )

        def attn_pair(b, p, pre_iter=None):
            require({("qk", b, p), ("v", b, 0 if p < 3 else 1)})
            qkT_t, v_t = qkT_ts[b], v_ts[b]
            for hf in range(2):
                po = [
                    ps_out.tile([HD + 1, 512], F32, tag="po",
                                name=f"po_{b}_{p}_{hf}_{hs}")
                    for hs in range(2)
                ]
                for kt in range(NKT):
                    if pre_iter is not None:
                        pre_iter(hf, kt)
                    sc = ps_sc.tile([P, 2, 512], F32, tag="sc")
                    for hs in range(2):
                        qo = hs * HD
                        nc.tensor.matmul(
                            sc[:, hs, :],
                            qkT_t[qo:qo + HD, KT + p, kt * P:(kt + 1) * P],
                            qkT_t[qo:qo + HD, p, hf * 512:(hf + 1) * 512],
                            start=True, stop=True,
                        )
                    ex = exp_p.tile([P, 2, 512], BF16, tag="ex")
                    nc.scalar.activation(ex[:], sc[:], EXP, scale=SCALE)

                    def mk_pv(hf=hf, kt=kt, ex=ex, po=po):
                        def f():
                            for hs in range(2):
                                nc.tensor.matmul(
                                    po[hs][:],
                                    v_t[:, kt, 2 * p + hs, :],
                                    ex[:, hs, :],
                                    start=(kt == 0), stop=(kt == NKT - 1),
                                )
                            if kt == NKT - 1:
                                epilogue(b, p, hf, po,
                                         fast=(b == 1 and p == NP - 1 and hf == 1))
                        return f
                    pv_q.append(mk_pv())
                    pump_pv()
                    run_fillers()

        # --- schedule ---
        # head: qk for pair 0, then v(j=0) halves dripped so PV(p0, kt)
        # (popped at iteration kt+2) always finds v0(rt=kt) already emitted
        for nt in (0, KT):
            for hf in range(2):
                emit_unit(qk_steps(0, nt, hf))
        emit_unit(v_steps(0, 0, 0))
        emit_unit(v_steps(0, 1, 0))

        def pre_iter_p0(hf, kt):
            if hf == 0 and kt < NKT - 2:
                emit_unit(v_steps(0, kt + 2, 0))

        # b0-window fillers in due-time order: qk(0,p) is consumed at pair p,
        # v0(j=1) at pair 3, b1's v(j=0)+qk(1,0) at b1 pair 0
        def push_qk(b, p_):
            push_unit(("qk", b, p_), qk_steps(b, p_, 0), MM_QK)
            push_unit(("qk", b, p_), qk_steps(b, KT + p_, 0), MM_QK)
            push_unit(("qk", b, p_), qk_steps(b, p_, 1), MM_QK)
            push_unit(("qk", b, p_), qk_steps(b, KT + p_, 1), MM_QK)

        push_qk(0, 1)
        push_qk(0, 2)
        for rt in range(NKT):
            push_unit(("v", 0, 1), v_steps(0, rt, 1), MM_VP)
        push_qk(0, 3)
        push_qk(0, 4)
        push_qk(0, 5)
        for rt in range(NKT):
            push_unit(("v", 1, 0), v_steps(1, rt, 0), MM_VP)
        push_qk(1, 0)

        for p in range(NP):
            attn_pair(0, p, pre_iter=pre_iter_p0 if p == 0 else None)

        # b1-window fillers in due-time order
        push_qk(1, 1)
        push_qk(1, 2)
        for rt in range(NKT):
            push_unit(("v", 1, 1), v_steps(1, rt, 1), MM_VP)
        push_qk(1, 3)
        push_qk(1, 4)
        push_qk(1, 5)
        for rt in range(NKT):
            push_unit(("proj", 0), proj_steps(0, rt), MM_VP)

        def pre_iter_last(hf, kt):
            # b1 projection first half inside the last attention window
            if hf == 1 and 2 <= kt <= 5:
                emit_unit(proj_steps(1, kt - 2))

        for p in range(NP):
            attn_pair(1, p, pre_iter=pre_iter_last if p == NP - 1 else None)
        pump_pv(force=True)
        force_drain()
        for rt in range(4, NKT):
            emit_unit(proj_steps(1, rt))

    nc.compile()
    return nc


_NC_CACHE = None


def _get_nc():
    global _NC_CACHE
    if _NC_CACHE is None:
        _NC_CACHE = build_kernel()
    return _NC_CACHE


def make_in_maps(x, W_qkv, W_proj, b_proj):
    x = np.asarray(x, np.float32)
    wq_full = np.asarray(W_qkv, np.float32)
    # q|k columns -> [p, nt, kt, c] layout, flattened to [128, 9216]
    wqk_r = wq_full[:, :2 * D].reshape(KT, P, 2 * KT, P).transpose(1, 2, 0, 3)
    wqk_host = np.ascontiguousarray(wqk_r.reshape(P, 2 * KT * KT * P)).astype(
        ml_dtypes.bfloat16)
    wv_host = np.ascontiguousarray(wq_full[:, 2 * D:]).astype(ml_dtypes.bfloat16)
    wp = np.asarray(W_proj, np.float32).astype(ml_dtypes.bfloat16)
    bias = np.ascontiguousarray(
        np.broadcast_to(np.asarray(b_proj, np.float32), (P, D))
    )
    in_maps = []
    for c in range(NCORES):
        xc = x[BPC * c:BPC * (c + 1)].reshape(ROWS, D).T
        in_maps.append({
            "xT": np.ascontiguousarray(xc).astype(ml_dtypes.bfloat16),
            "wqk": wqk_host, "wv": wv_host, "wproj": wp, "bias": bias,
        })
    return in_maps


def run(x, W_qkv, W_proj, b_proj, trace=False):
    nc = _get_nc()
    in_maps = make_in_maps(x, W_qkv, W_proj, b_proj)
    res = run_bass_kernel_spmd(nc, in_maps, core_ids=list(range(NCORES)), trace=trace)
    y = np.concatenate(
        [res.results[c]["out"].reshape(BPC, N, D) for c in range(NCORES)], axis=0
    )
    return y.astype(np.float32), res


def kernel(x, W_qkv, W_proj, b_proj):
    y, _ = run(x, W_qkv, W_proj, b_proj, trace=False)
    return y
